# revision 1
# baseline (speedup 1.0000x reference)
"""MaxMarginCriterion loss on 8 TRN2 NeuronCores (Bass/Tile).

reference:
    correct_sim[r] = cossim[r, argmax(target[r])]
    loss = mean_r( sum_c( relu(MARGIN + cossim - correct_sim) * (1 - target) ) )

Identity used on-device (target is exactly one-hot, so cossim[r, correct] ==
correct_sim[r] exactly in the rounded dtype cossim is stored in, and the
correct column contributes relu(MARGIN) == MARGIN to the unmasked sum):
    row_sum[r] = sum_c relu(MARGIN + cossim[r, c] - correct_sim[r])
    loss = (sum_r row_sum[r] - MARGIN * N) / N

Sharding: data-parallel over the batch axis — core k handles rows
[k*2048, (k+1)*2048). Each core computes per-partition partial sums
(output [128, 16]); the final reduction over 8*128*16 floats happens on
host (the "all-reduce mean" of the sharding hint).

The problem is memory-bound. The previous version moved the full
f32+int64 inputs (48 MiB/core) and ran at that traffic's HBM roofline
(~148.6 us). The on-device representation is chosen at sharding time:
    cossim -> float16 (8 MiB/core; loss rel err ~1e-6 vs 2e-2 tol,
              rounding averages out over 16384 rows)
    target -> NEGATED int8 one-hot (4 MiB/core; exact for 0/1 values)
cutting per-core traffic to 12 MiB. Measured wall ~38 us/pass with the
three engines nearly balanced: DVE ~40 us busy at the wall (int8 holds
scalar_tensor_tensor at 1x mode), ACT ~33 us, DMA ~34 us (373 GB/s/core
with 4-block chunked DMAs, io_bufs=4 deep; io_bufs=2 starves DMA and
costs +13 us). Rejected on measurement: fp8 cossim (modeled ~32 us,
measured 45 — fp8 operand reads are far slower than the cost model
claims), fp16-target blocks for DVE 2x mode (monotonically worse:
nd=16/12/8 -> 38.0/39.2/50.1 us — the 2x mode does not materialize with
a real DMA'd second stream), and relu-on-DVE blocks (+5 us straight on
the DVE critical path).

Per 128-row block on device (DMA in 4-block chunks of 2 MiB + 1 MiB):
    DVE  scalar_tensor_tensor: prod = (cos - MARGIN) * tneg, accum_out
         -> bias = MARGIN - correct_sim   (the negated one-hot folds the
         "MARGIN - corr" affine into the one reduction op)
    ACT  activation Relu(cos + bias), accum_out -> acc[:, i]

(tensor_tensor_reduce is avoided: its TENSOR_TENSOR_REDUCE opcode wedges
the exec unit on this runtime; scalar_tensor_tensor with accum_out does
the same fused multiply+row-sum and runs fine. tensor_scalar with an AP
scalar silently drops op1/accum_out — do not use it for relu.)
"""

import time

import numpy as np

import concourse.bacc as bacc
import concourse.tile as tile
from concourse import mybir
from concourse.bass_utils import run_bass_kernel_spmd

MARGIN = 0.1
N, C = 16384, 2048
NCORES = 8
ROWS = N // NCORES        # rows per core
P = 128                   # SBUF partitions
NT = ROWS // P            # 128-row blocks per core
BLK = 4                   # blocks per DMA chunk

# Per-core block classes (cossim is fp16 everywhere):
#   blocks [0, ND):        target int8   — DVE corr at 1x, relu on ACT
#   blocks [ND, NT-NB):    target fp16   — DVE corr at 2x, relu on ACT
#   blocks [NT-NB, NT):    target fp16   — DVE corr at 2x, relu on DVE
# ND trades DMA bytes (fp16 target = 2x int8) against DVE cycles (2x mode
# needs all-16-bit operands); NB offloads ACT onto leftover DVE slack.
# ND must be a multiple of BLK. Measured (one process, 1024-pass hw-loop
# differencing): nd=16 38.0us < nd=12 39.2 < nd=12,nb=1 44.0 < nd=8 50.1 —
# the fp16-target 2x mode does NOT pay off with a real DMA'd stream, and
# DVE-relu blocks add straight onto the DVE critical path. Uniform int8
# target wins; keep ND=NT, NB=0.
ND, NB = 16, 0

_NC_CACHE = {}


def _build(reps=1, hw_loop_iters=0, nd=ND, nb=NB, blk=BLK,
           io_bufs=4, work_bufs=8, corr_op="stt"):
    """One NEFF doing `reps` python-unrolled full passes over the inputs.
    If hw_loop_iters > 0, wrap the passes in a tc.For_i hardware loop
    executing hw_loop_iters times (for high-rep timing without giant
    NEFFs); total passes = reps * hw_loop_iters."""
    assert nd % blk == 0 and 0 <= nb <= NT - nd
    nf = NT - nd              # fp16-target blocks
    n_act = NT - nb           # blocks whose relu runs on ACT
    nch = NT // blk
    nc = bacc.Bacc("TRN2", target_bir_lowering=False, debug=False)
    # [NT, P, C] is the same row-major bytes as [ROWS, C]
    cos = nc.dram_tensor("cossim", [NT, P, C], mybir.dt.float16, kind="ExternalInput").ap()
    t8 = t16 = None
    if nd:
        t8 = nc.dram_tensor("t8", [nd, P, C], mybir.dt.int8, kind="ExternalInput").ap()
    if nf:
        t16 = nc.dram_tensor("t16", [nf, P, C], mybir.dt.float16, kind="ExternalInput").ap()
    out = nc.dram_tensor("out", [P, NT], mybir.dt.float32, kind="ExternalOutput").ap()

    with tile.TileContext(nc) as tc:
        with (
            tc.tile_pool(name="io", bufs=io_bufs) as io_pool,
            tc.tile_pool(name="work", bufs=work_bufs) as work,
            tc.tile_pool(name="accp", bufs=1) as accp,
        ):
            acc_act = accp.tile([P, n_act], mybir.dt.float32, tag="acc_act")
            acc_dve = None
            z16 = None
            if nb:
                acc_dve = accp.tile([P, nb], mybir.dt.float32, tag="acc_dve")
                z16 = accp.tile([P, C], mybir.dt.float16, tag="z16")
                nc.vector.memset(z16, 0.0)

            def one_pass():
                for ch in range(nch):
                    lo, hi = ch * blk, (ch + 1) * blk
                    cos_t = io_pool.tile([P, blk, C], mybir.dt.float16, tag="cos")
                    nc.sync.dma_start(
                        out=cos_t, in_=cos[lo:hi].rearrange("b p c -> p b c"))
                    if hi <= nd:
                        tgt_t = io_pool.tile([P, blk, C], mybir.dt.int8, tag="t8")
                        nc.sync.dma_start(
                            out=tgt_t, in_=t8[lo:hi].rearrange("b p c -> p b c"))
                    else:
                        tgt_t = io_pool.tile([P, blk, C], mybir.dt.float16, tag="t16")
                        nc.sync.dma_start(
                            out=tgt_t,
                            in_=t16[lo - nd:hi - nd].rearrange("b p c -> p b c"))
                    for b in range(blk):
                        i = lo + b
                        cos_b = cos_t[:, b, :]
                        prod = work.tile([P, C], mybir.dt.float16, tag="prod")
                        bias = work.tile([P, 1], mybir.dt.float32, tag="bias")
                        # prod = (cos - MARGIN) * tneg; bias = MARGIN - corr
                        if corr_op == "stt":
                            nc.vector.scalar_tensor_tensor(
                                out=prod, in0=cos_b, scalar=-MARGIN,
                                in1=tgt_t[:, b, :],
                                op0=mybir.AluOpType.add, op1=mybir.AluOpType.mult,
                                accum_out=bias,
                            )
                        else:       # same math via the custom DVE op
                            nc.vector.affine_mul_reduce(
                                out=prod, accum_out=bias, in0=cos_b,
                                in1=tgt_t[:, b, :], scale=1.0, bias=-MARGIN,
                            )
                        relu = work.tile([P, C], mybir.dt.float16, tag="relu")
                        if i < n_act:
                            nc.scalar.activation(
                                out=relu, in_=cos_b,
                                func=mybir.ActivationFunctionType.Relu,
                                bias=bias, scale=1.0,
                                accum_out=acc_act[:, i:i + 1],
                            )
                        else:
                            # (cos + bias) max 0 = relu(cos + bias) on DVE
                            j = i - n_act
                            nc.vector.scalar_tensor_tensor(
                                out=relu, in0=cos_b, scalar=bias, in1=z16,
                                op0=mybir.AluOpType.add, op1=mybir.AluOpType.max,
                                accum_out=acc_dve[:, j:j + 1],
                            )

            if hw_loop_iters > 0:
                with tc.For_i(0, hw_loop_iters):
                    for _ in range(reps):
                        one_pass()
            else:
                for _ in range(reps):
                    one_pass()
            nc.sync.dma_start(out=out[:, 0:n_act], in_=acc_act)
            if nb:
                nc.sync.dma_start(out=out[:, n_act:NT], in_=acc_dve)
    nc.compile()
    return nc


def _get_nc():
    if "nc" not in _NC_CACHE:
        _NC_CACHE["nc"] = _build()
    return _NC_CACHE["nc"]


def _prep_inputs(cossim, target, nd=ND):
    """Host-side representation change done while sharding: cossim f32 ->
    f16; one-hot int64 target -> negated int8 (first nd blocks per core)
    and negated fp16 (rest). Returns full arrays shaped [NCORES*n, P, C]
    (contiguous per-core along axis 0)."""
    nf = NT - nd
    cos16 = np.ascontiguousarray(np.asarray(cossim), dtype=np.float16)
    t = np.asarray(target).reshape(NCORES, NT, P, C)
    out = {"cossim": cos16.reshape(NCORES * NT, P, C)}
    if nd:
        t8 = np.ascontiguousarray(t[:, :nd]).astype(np.int8)
        np.negative(t8, out=t8)     # -1 at the correct column, exact
        out["t8"] = t8.reshape(NCORES * nd, P, C)
    if nf:
        t16 = np.ascontiguousarray(t[:, nd:]).astype(np.float16)
        np.negative(t16, out=t16)   # -1.0 at the correct column, exact
        out["t16"] = t16.reshape(NCORES * nf, P, C)
    return out


def _run(cossim, target, trace=False, trace_kwargs=None):
    full = _prep_inputs(cossim, target)
    nf = NT - ND
    nc = _get_nc()
    in_maps = []
    for k in range(NCORES):
        m = {"cossim": full["cossim"][k * NT:(k + 1) * NT]}
        if ND:
            m["t8"] = full["t8"][k * ND:(k + 1) * ND]
        if nf:
            m["t16"] = full["t16"][k * nf:(k + 1) * nf]
        in_maps.append(m)
    # The shared device occasionally starts wedged from a prior tenant
    # (NRT_EXEC_UNIT_UNRECOVERABLE / "mesh desynced") and recovers within
    # ~a minute; retry rather than fail the whole call. Non-transient
    # errors (bad imports, shape/type bugs) re-raise immediately.
    for attempt in range(3):
        try:
            res = run_bass_kernel_spmd(
                nc, in_maps, core_ids=list(range(NCORES)),
                trace=trace, **(trace_kwargs or {}),
            )
            break
        except (ImportError, AssertionError, TypeError, ValueError, KeyError):
            raise
        except Exception:  # jax.errors.JaxRuntimeError et al.
            if attempt == 2:
                raise
            time.sleep(60)
    total = 0.0
    for k in range(NCORES):
        total += res.results[k]["out"].sum(dtype=np.float64)
    loss = (total - MARGIN * N) / N
    return np.asarray(loss, dtype=np.float32), res


def kernel(cossim, target):
    loss, _ = _run(cossim, target)
    return loss

